# revision 1
# baseline (speedup 1.0000x reference)
"""Trainium2 Bass kernel for nn_Net_53644141527184.

Computation (per batch b):
  For each branch br in {x1, x3, x5}:
    picked[b, g, p] = x_br[b, idx[br, g, p], p]          (channel gather, p = 0..255)
    grid = picked.reshape(B, 128, 16, 16)
    crop[b, g, i, j] = grid[b, g, oh[g]+i, ow[g]+j]      (per-group 14x14 crop)
  feats = concat(crops, axis=1)                          -> [B, 384, 14, 14]
  out = einsum('bchw,oc->bohw', feats, W)                -> [B, 768, 14, 14]

Strategy: pure data parallel over 8 NeuronCores (16 batches each).
x is cast to bf16 host-side (the conv consumes bf16 anyway, so results are
bit-identical to casting on device) and relaid out to
[ko=8, (p_local 8 x b 16)=128, (br 3, ki 4, c 512)] so each SBUF tile has
partition = (position, batch) with 16-partition groups = one grid position.

Groups g are stable-sorted host-side by crop offset v = 2*oh+ow and assigned
to a 32-aligned padded slot layout (pad slots gather channel 0 and carry
zero W rows, so they contribute nothing).  This makes every crop band start
at a 32-aligned partition, which the vector/scalar engines require.

Per core pipeline:
  1. DMA one x tile [128, 3*4*512] bf16 per ko (12KB contiguous runs).
  2. gpsimd.indirect_copy with one fused index list per ko (indices
     pre-offset by branch/ki host-side): per-16-partition-group index lists
     = per-position channel picks shared by the 16 batches of the group
     -> picked [(p_local, b), 3*4*S slots] bf16.
  3. PE transpose (bf16 identity) per 128-slot chunk; the two k-blocks of a
     grid row land in one PSUM tile [slots, (k2 2, pl 8, b 16)] so a crop
     band spans a full 14-wide output row.
  4. Crop fused into the PSUM->SBUF copy: per (band, grid row), one strided
     DVE/ACT copy moves [slots, 14 pl', 16 b] straight into the conv-K tiles
     feats[half][tile][slot, (b 16, q 98)] bf16.  Residual slots (>128) of
     all branches pack into shared extra conv-K tiles.
  5. 1x1 conv: out[o_chunk, (16 b x 98 q)] = sum over conv-K tiles of
     WT_tile^T @ feats_tile (bf16 matmuls, f32 PSUM, N=392 into 2-bank PSUM
     tiles), copied to SBUF bf16, DMA'd out with 3136B contiguous runs.
Output returns as bf16 and is upcast host-side (adds < 0.4% quantization,
well inside the 2e-2 gate).  Index arrays / W / x are preprocessed
host-side into device-friendly layouts (pure relayout + index address
arithmetic; all data-dependent movement happens on device).
"""

import numpy as np
import ml_dtypes
from contextlib import ExitStack

import concourse.bacc as bacc
import concourse.bass as bass
import concourse.tile as tile
import concourse.mybir as mybir
from concourse import bass_utils, masks

N_CORES = 8
B = 16        # batches per core
C = 512
P = 256       # grid positions (16x16)
G = 128       # groups per branch
NQ = 196      # cropped positions (14x14)
BR = 3
OC = 768
NK = 32       # position blocks of 8 (k = 2*row + col_half)
KB = 4        # k-blocks per x DMA tile / gather
NKO = NK // KB

_CACHE = {}


ALIGN = 32   # crop-band partition alignment (engine partition bases must be
             # 0/32/64/96, so bands must start on the 32 grid)


def _plan(offh, offw):
    """Compute the padded slot layout and conv-tile packing."""
    A = ALIGN
    v = 2 * offh.astype(int) + offw.astype(int)
    perms = [np.argsort(v[br], kind="stable") for br in range(BR)]
    plan = {"perms": perms, "S": [], "bands": [], "pieces": []}
    for br in range(BR):
        cnt = np.bincount(v[br], minlength=4)
        slot = 0
        bands = []
        for vv in range(4):
            n = int(cnt[vv])
            bands.append((vv, slot, n))
            slot += ((n + A - 1) // A) * A
        S = max(slot, 128)
        if S % 16:
            S += 16 - (S % 16)
        plan["S"].append(S)
        plan["bands"].append(bands)

    # residual chunks (slots >= 128) pack greedily into extra tiles
    resid_assign = {}
    bins = []
    for br in range(BR):
        sz = plan["S"][br] - 128
        sz = ((sz + A - 1) // A) * A
        if sz <= 0:
            continue
        placed = False
        for i in range(len(bins)):
            if bins[i] + sz <= 128:
                resid_assign[br] = (BR + i, bins[i])
                bins[i] += sz
                placed = True
                break
        if not placed:
            bins.append(sz)
            resid_assign[br] = (BR + len(bins) - 1, 0)
    n_tiles = BR + len(bins)
    plan["n_tiles"] = n_tiles

    # copy pieces: band slot sub-ranges -> (tile, tile partition offset).
    # Copies are extended over the 32-alignment pad rows (their data is a
    # harmless finite duplicate; their W rows are zero), which leaves no
    # unwritten rows below each tile's used extent and costs nothing (engine
    # copy time depends only on the free-dim size).
    # Engine partition windows are buddy-aligned: from base b != 0 an access
    # must not cross the b + (b & -b) boundary; base 0 is unrestricted.
    used_rows = [0] * n_tiles
    for br in range(BR):
        for (vv, slot_lo, n) in plan["bands"][br]:
            lo = slot_lo
            remaining = ((n + A - 1) // A) * A
            while remaining > 0:
                chunk = lo // 128
                in_chunk = lo % 128
                take = min(remaining, 128 - in_chunk)
                if chunk == 0:
                    tid, tofs = br, in_chunk
                else:
                    tid, base = resid_assign[br]
                    tofs = base + in_chunk
                off = 0
                while off < take:
                    b = tofs + off
                    lim = take - off if b == 0 else min(take - off,
                                                        (b & -b))
                    plan["pieces"].append((br, vv, tid, tofs + off,
                                           lo + off, lim))
                    off += lim
                used_rows[tid] = max(used_rows[tid], tofs + take)
                lo += take
                remaining -= take
    plan["used_rows"] = used_rows

    # W rows per tile: tile partition row -> (br, original g) or None
    rows = [[None] * 128 for _ in range(n_tiles)]
    for br in range(BR):
        pos = 0
        for (vv, slot_lo, n) in plan["bands"][br]:
            for i in range(n):
                s = slot_lo + i
                g_orig = int(perms[br][pos + i])
                chunk = s // 128
                if chunk == 0:
                    tid, tofs = br, s
                else:
                    tid, base = resid_assign[br]
                    tofs = base + (s % 128)
                rows[tid][tofs] = (br, g_orig)
            pos += n
    plan["tile_rows"] = rows
    return plan


# output-row groups per q-half: (first row in half, n rows); a conv group
# unlocks once the grid rows it reads (max qr + 1, for the dh=1 band) have
# been gathered and cropped.  The first half leads with single-row groups so
# conv matmuls start ~one ko earlier -- the PE is idle early and saturated
# late, so pulling conv work forward shortens the tail.
QGRPS_H = [
    [(0, 1), (1, 1), (2, 2), (4, 2), (6, 1)],
    [(0, 2), (2, 2), (4, 2), (6, 1)],
]


def _build_program(plan):
    nc = bacc.Bacc("TRN2", target_bir_lowering=False, debug=False,
                   num_devices=N_CORES)

    S = plan["S"]
    n_tiles = plan["n_tiles"]
    TS = KB * sum(S)          # gathered slots per ko (all branches, all ki)

    f32 = mybir.dt.float32
    bf16 = mybir.dt.bfloat16

    x_d = nc.dram_tensor("x", [NKO, BR, 128, KB * C], bf16,
                         kind="ExternalInput")
    # index blocks per (ko, br): KB*S[br] gather indices into the branch's
    # [KB*C] x tile (HW IndirectCopy caps dst elems at 1024)
    idxt_d = nc.dram_tensor("idxt", [128, NKO * (TS // 16)],
                            mybir.dt.uint16, kind="ExternalInput")
    wt_d = nc.dram_tensor("wt", [128, n_tiles * OC], bf16,
                          kind="ExternalInput")
    # group-major flat output [h][g][oc][b][qg]: one small DMA per
    # (half, group, oc-chunk) fires as soon as that conv group is drained,
    # so the kernel tail is just the last row group; host reassembles
    out_d = nc.dram_tensor("out", [2 * OC * B * 98], bf16,
                           kind="ExternalOutput")

    # base slot offset of (br, ki) inside the per-ko gather output
    def slot_base(br, ki):
        return KB * sum(S[:br]) + ki * S[br]

    pieces_by_brc = {}
    for br, vv, tid, tofs, slot_lo, n in plan["pieces"]:
        pieces_by_brc.setdefault((br, slot_lo // 128), []).append(
            (vv, tid, tofs, slot_lo % 128, n))

    with tile.TileContext(nc) as tc, ExitStack() as ctx:
        cpool = ctx.enter_context(tc.tile_pool(name="const", bufs=1))
        xpool = ctx.enter_context(tc.tile_pool(name="xin", bufs=8))
        ppool = ctx.enter_context(tc.tile_pool(name="picked", bufs=3))
        featp = ctx.enter_context(tc.tile_pool(name="feats", bufs=1))
        opool = ctx.enter_context(tc.tile_pool(name="ostage", bufs=6))
        t2p = ctx.enter_context(tc.tile_pool(name="ps_t2", bufs=5, space="PSUM"))
        cvp = ctx.enter_context(tc.tile_pool(name="ps_cv", bufs=3, space="PSUM"))

        identb = cpool.tile([128, 128], bf16)
        masks.make_identity(nc, identb[:])
        idxt = cpool.tile([128, NKO * (TS // 16)], mybir.dt.uint16)
        nc.sync.dma_start(idxt[:, :TS // 16], idxt_d.ap()[:, :TS // 16])
        nc.sync.dma_start(idxt[:, TS // 16:], idxt_d.ap()[:, TS // 16:])
        wtb = cpool.tile([128, n_tiles * OC], bf16)

        # feats split per (half, conv tile, row group) so each conv group
        # reads a fully-written tile and later crop writes never carry a
        # write-after-read dependency on earlier conv reads.  Free layout is
        # q-major (q, b) so crop copies have packed 2-byte last dims on both
        # sides and run in the DVE 2x mode.  Crops go exclusively to DVE and
        # conv drains to ACT so a conv copy waiting on its matmuls never
        # blocks later crops in the same engine FIFO.
        feats = [[[featp.tile([128, B * 14 * ng, ], bf16,
                              name=f"feat{h}_{i}_{g}")
                   for g, (_, ng) in enumerate(QGRPS_H[h])]
                  for i in range(n_tiles)] for h in range(2)]

        # only rows above each tile's written extent need zeroing (W rows
        # for pad slots are zero, so written pad rows are harmless)
        for h in range(2):
            for i in range(n_tiles):
                for g in range(len(QGRPS_H[h])):
                    u = plan["used_rows"][i]
                    while u < 128:
                        span = 128 - u if u == 0 else min(128 - u, u & -u)
                        nc.vector.memset(feats[h][i][g][u:u + span, :], 0.0)
                        u += span

        def conv_group(h, g, ko_ready):
            q0, ng = QGRPS_H[h][g]
            N = B * 14 * ng
            gbase = OC * B * (98 * h + 14 * q0)
            for oc in range(6):
                pc = cvp.tile([128, N], f32)
                for t in range(n_tiles):
                    lhsT = wtb[:, t * OC + oc * 128:
                               t * OC + (oc + 1) * 128]
                    nc.tensor.matmul(pc[:], lhsT, feats[h][t][g][:],
                                     start=(t == 0),
                                     stop=(t == n_tiles - 1))
                ot = opool.tile([128, N], bf16)
                nc.scalar.copy(ot[:], pc[:])
                start = gbase + oc * 128 * N
                dd = out_d.ap()[start:start + 128 * N].rearrange(
                    "(o x) -> o x", o=128)
                nc.sync.dma_start(dd, ot[:])

        # conv groups become ready once the grid rows they read are cropped:
        # group (h, g) reads qr in [7h+q0, 7h+q0+ng) and needs rows up to
        # max qr + 1, i.e. all kos through (max qr + 1) // 2
        ready = {}
        for h in range(2):
            for g, (q0, ng) in enumerate(QGRPS_H[h]):
                ready.setdefault((7 * h + q0 + ng) // 2, []).append((h, g))

        for ko in range(NKO):
            # grid row 15 (ki 2,3 of the last ko) is never read by any crop;
            # skip its input, gather, and transposes
            nki = KB if ko < NKO - 1 else KB // 2
            xts = []
            for br in range(BR):
                xt = xpool.tile([128, KB * C], bf16)
                nc.sync.dma_start(xt[:, :nki * C],
                                  x_d.ap()[ko, br][:, :nki * C])
                xts.append(xt)
            pk = ppool.tile([128, TS], bf16)
            icol = ko * (TS // 16)
            for br in range(BR):
                ns = nki * S[br]
                ob = KB * sum(S[:br])
                nc.gpsimd.indirect_copy(
                    pk[:, ob:ob + ns], xts[br][:, :nki * C],
                    idxt[:, icol + ob // 16:icol + (ob + ns) // 16],
                    i_know_ap_gather_is_preferred=True)
            if ko == 0:
                nc.sync.dma_start(wtb[:], wt_d.ap())
            for (h, g) in ready.get(ko - 1, []):
                conv_group(h, g, ko - 1)
            for br in range(BR):
                sb = S[br]
                nch = (sb + 127) // 128
                # all 8 transposed blocks of this (ko, br) share one 1-bank
                # PSUM tile: col block (chunk, r2, k2) -> [rows, (pl, b)]
                pt = t2p.tile([128, 1024], bf16)
                for r2 in range(nki // 2):
                    for chunk in range(nch):
                        cn = min(128, sb - 128 * chunk)
                        for k2 in range(2):
                            off = (slot_base(br, 2 * r2 + k2) + 128 * chunk)
                            nc.tensor.transpose(
                                pt[:cn, chunk * 512 + r2 * 256 + k2 * 128:
                                   chunk * 512 + r2 * 256 + (k2 + 1) * 128],
                                pk[:, off:off + cn], identb[:])
                for vv in range(4):
                    dh, dw = vv // 2, vv % 2
                    # can rows r2=0,1 (qr, qr+1) merge into one copy?
                    info = []
                    for r2 in range(nki // 2):
                        qr = ko * 2 + r2 - dh
                        if not (0 <= qr < 14):
                            info.append(None)
                            continue
                        qh, qrh = qr // 7, qr % 7
                        g = next(i for i, (q0, ng) in enumerate(QGRPS_H[qh])
                                 if q0 <= qrh < q0 + ng)
                        info.append((qh, g, qrh - QGRPS_H[qh][g][0]))
                    merged = (len(info) == 2 and None not in info
                              and info[0][:2] == info[1][:2]
                              and info[0][2] == 0 and info[1][2] == 1)
                    for chunk in range(nch):
                        for (vv_, tid, tofs, plo, n) in \
                                pieces_by_brc.get((br, chunk), []):
                            if vv_ != vv:
                                continue
                            pv = pt[plo:plo + n].rearrange(
                                "g (ch r2 pl b) -> g ch r2 pl b",
                                ch=2, r2=2, pl=16)
                            if merged:
                                qh, g, _ = info[0]
                                src = pv[:, chunk, :, dw:dw + 14, :]
                                dst = feats[qh][tid][g][
                                    tofs:tofs + n].rearrange(
                                    "g (qr q b) -> g qr q b",
                                    qr=2, q=14)
                                nc.vector.tensor_copy(dst, src)
                                continue
                            for r2 in range(nki // 2):
                                if info[r2] is None:
                                    continue
                                qh, g, qrow = info[r2]
                                # valid grid cols pl' span [dw, 14+dw)
                                src = pv[:, chunk, r2, dw:dw + 14, :]
                                dst = feats[qh][tid][g][
                                    tofs:tofs + n].rearrange(
                                    "g (q b) -> g q b", b=B)[
                                    :, 14 * qrow:14 * (qrow + 1), :]
                                nc.vector.tensor_copy(dst, src)
        for (h, g) in ready.get(NKO - 1, []):
            conv_group(h, g, NKO - 1)

    nc.compile()
    return nc


def _prep_aux(idx, offh, offw, W, plan):
    """Host-side index/layout preprocessing (relayout + address arithmetic)."""
    idx = np.asarray(idx)
    W = np.asarray(W, dtype=np.float32)
    perms = plan["perms"]
    S = plan["S"]
    TS = KB * sum(S)

    # padded sorted index array per branch: [S[br], 256]
    idx_pad = [np.zeros((S[br], P), np.int64) for br in range(BR)]
    for br in range(BR):
        pos = 0
        for (vv, slot_lo, n) in plan["bands"][br]:
            idx_pad[br][slot_lo:slot_lo + n] = idx[br][perms[br][pos:pos + n]]
            pos += n

    # gather index lists per (ko, br), concatenated: u[j] for j = (br, ki, s)
    # = element offset ki*C + idx_pad[br, s, p(k=ko*KB+ki, pl)] into the
    # branch's [KB*C] slice of the x tile.
    # storage: idxt[16*pl + (j%16), ko*(TS//16) + j//16]  (branch blocks are
    # 16-aligned so per-block wrapping matches the global reshape)
    u = np.zeros((NKO, 8, TS), np.int64)
    for br in range(BR):
        for ki in range(KB):
            for ko in range(NKO):
                k = ko * KB + ki
                r, c0 = k // 2, 8 * (k % 2)
                pl = np.arange(8)
                p = 16 * r + c0 + pl                    # [8]
                base = KB * sum(S[:br]) + ki * S[br]
                vals = ki * C + idx_pad[br][:, p].T     # [8 pl, S]
                u[ko, :, base:base + S[br]] = vals
    t = u.reshape(NKO, 8, TS // 16, 16)                 # [ko, pl, col, r16]
    t = t.transpose(1, 3, 0, 2)                         # [pl, r16, ko, col]
    idxt = np.ascontiguousarray(
        t.reshape(128, NKO * (TS // 16))).astype(np.uint16)

    # W tiles: [g row, tile, o]; zero rows for pad slots
    Wr = W.reshape(OC, BR, 128)                         # [o, br, g]
    n_tiles = plan["n_tiles"]
    wt = np.zeros((128, n_tiles, OC), np.float32)
    for tid in range(n_tiles):
        for row in range(128):
            ent = plan["tile_rows"][tid][row]
            if ent is not None:
                br, g_orig = ent
                wt[row, tid] = Wr[:, br, g_orig]
    wt = np.ascontiguousarray(
        wt.reshape(128, n_tiles * OC)).astype(ml_dtypes.bfloat16)
    return idxt, wt


def _relayout_x(xs):
    """3 x [16, 512, 256] f32 -> [8, 3, 128, 4*512] bf16:
    out[ko, br, pl*16+b, ki*512+c] = x_br[b, c, 8*(4*ko+ki)+pl]."""
    stack = np.stack([x.reshape(B, C, NK, 8) for x in xs])  # [br, b, c, k, pl]
    t = stack.reshape(BR, B, C, NKO, KB, 8)
    t = t.transpose(3, 0, 5, 1, 4, 2)               # [ko, br, pl, b, ki, c]
    t = np.ascontiguousarray(t.reshape(NKO, BR, 128, KB * C))
    return t.astype(ml_dtypes.bfloat16)


def kernel(x1, x3, x5, W, idx, offh, offw):
    x1 = np.asarray(x1, dtype=np.float32)
    x3 = np.asarray(x3, dtype=np.float32)
    x5 = np.asarray(x5, dtype=np.float32)
    Bfull = x1.shape[0]
    assert Bfull == N_CORES * B

    offh = np.asarray(offh).astype(np.int64)
    offw = np.asarray(offw).astype(np.int64)
    plan = _plan(offh, offw)
    idxt, wt = _prep_aux(idx, offh, offw, W, plan)

    key = (tuple(plan["S"]), plan["n_tiles"],
           tuple(plan["pieces"]))
    if _CACHE.get("key") != key:
        _CACHE["nc"] = _build_program(plan)
        _CACHE["key"] = key
    nc = _CACHE["nc"]

    in_maps = []
    for core in range(N_CORES):
        sl = slice(core * B, (core + 1) * B)
        in_maps.append({
            "x": _relayout_x([x1[sl].reshape(B, C, P),
                              x3[sl].reshape(B, C, P),
                              x5[sl].reshape(B, C, P)]),
            "idxt": idxt,
            "wt": wt,
        })

    res = bass_utils.run_bass_kernel_spmd(nc, in_maps, list(range(N_CORES)))
    outs = []
    for i in range(N_CORES):
        flat = np.asarray(res.results[i]["out"]).astype(np.float32)
        o = np.empty((B, OC, 14, 14), np.float32)
        pos = 0
        for h in range(2):
            for (q0, ng) in QGRPS_H[h]:
                blk = flat[pos:pos + OC * B * 14 * ng].reshape(
                    OC, ng, 14, B)
                o[:, :, 7 * h + q0:7 * h + q0 + ng, :] = blk.transpose(
                    3, 0, 1, 2)
                pos += OC * B * 14 * ng
        outs.append(o)
    return np.concatenate(outs, axis=0)



# revision 4
# speedup vs baseline: 2.4175x; 2.4175x over previous
"""Trainium2 Bass kernel for nn_Net_53644141527184.

Computation (per batch b):
  For each branch br in {x1, x3, x5}:
    picked[b, g, p] = x_br[b, idx[br, g, p], p]          (channel gather, p = 0..255)
    grid = picked.reshape(B, 128, 16, 16)
    crop[b, g, i, j] = grid[b, g, oh[g]+i, ow[g]+j]      (per-group 14x14 crop)
  feats = concat(crops, axis=1)                          -> [B, 384, 14, 14]
  out = einsum('bchw,oc->bohw', feats, W)                -> [B, 768, 14, 14]

Strategy: shard the 196 output positions q across the 8 cores (25 per core,
core 7 re-computing 4 of core 6's); every core handles ALL 128 batches and
ALL 768 output channels for its q-range.

x is relaid out host-side (data-independent transpose) to xg[br, p, c, b]
bf16, so the batch vector of one (channel, position) pick is one 256B
contiguous run in HBM.  The channel gather, the per-group crop, and the
transpose to matmul layout then all collapse into a single device-side
gpsimd.dma_gather per (branch, q-block): descriptor j = q_local*128 + g
fetches row p_local*512 + idx[br, g, p] of the core's 4-grid-row window
(p = 16*(oh[g]+qi) + (ow[g]+qj), all host-computed index arithmetic) and
lands it at partition g, row q_local of the feats tile [128 g, nq, 128 b].
Only the picked channels ever leave HBM (2.5 MB/core vs 12.6 MB for full x).

Conv: per (q-chunk of <=4, oc chunk of 128) one PSUM tile [128 o, 512] f32
accumulates 3 bf16 matmuls (one per branch tile; contraction is exactly
3 x 128, no padding).  PSUM drains alternate between the Activation and DVE
engines into per-(oc, q-block) staging tiles (bf16), each DMA'd out as soon
as its last chunk lands (runs >= 1KB).  Output returns bf16 [768, 25*128]
per core and is upcast/reassembled host-side (same quantization as the
conv's bf16 inputs; rel err ~4e-3, well inside the 2e-2 gate).
"""

import numpy as np
import ml_dtypes
from contextlib import ExitStack

import concourse.bacc as bacc
import concourse.bass as bass
import concourse.tile as tile
import concourse.mybir as mybir
from concourse import bass_utils

N_CORES = 8
B = 128       # batches (all on every core)
C = 512
G = 128       # groups per branch
BR = 3
OC = 768
NQ = 196      # output positions (14x14)
NQC = 25      # q positions per core
WROWS = 4     # grid rows in each core's x window
WP = 16 * WROWS                 # positions in window
Q0 = [25 * c for c in range(7)] + [NQ - NQC]   # per-core q-range start
QBLOCKS = [8, 8, 5, 4]          # gather/output q-blocks (sum = NQC);
                                # <= 8 q per block: the device-side SWDGE ring
                                # caps one dma_gather at 1024 descriptors
# PSUM accumulation chunks (offset, size) within each q-block
QCHUNKS = [[(0, 4), (4, 4)], [(0, 4), (4, 4)], [(0, 4), (4, 1)], [(0, 4)]]
NIDX_COLS = sum(BR * (qb * 128) // 16 for qb in QBLOCKS)   # 600

_CACHE = {}


def _build_program():
    nc = bacc.Bacc("TRN2", target_bir_lowering=False, debug=False,
                   num_devices=N_CORES, dynamic_dma_scratch_size=65536)

    f32 = mybir.dt.float32
    bf16 = mybir.dt.bfloat16

    xg_d = nc.dram_tensor("xg", [BR, WP * C, B], bf16, kind="ExternalInput")
    idxt_d = nc.dram_tensor("idxt", [128, NIDX_COLS], mybir.dt.int16,
                            kind="ExternalInput")
    wt_d = nc.dram_tensor("wt", [128, BR * OC], bf16, kind="ExternalInput")
    out_d = nc.dram_tensor("out", [OC, NQC * B], bf16, kind="ExternalOutput")

    with tile.TileContext(nc) as tc, ExitStack() as ctx:
        cpool = ctx.enter_context(tc.tile_pool(name="const", bufs=1))
        fpool = ctx.enter_context(tc.tile_pool(name="feats", bufs=1))
        opool = ctx.enter_context(tc.tile_pool(name="ostage", bufs=1))
        psump = ctx.enter_context(tc.tile_pool(name="ps", bufs=6, space="PSUM"))

        idxt = cpool.tile([128, NIDX_COLS], mybir.dt.int16)
        nc.sync.dma_start(idxt[:], idxt_d.ap())
        wtb = cpool.tile([128, BR * OC], bf16)
        nc.sync.dma_start(wtb[:], wt_d.ap())

        # gathers: q-block major, branch minor, so the 3 branch tiles of a
        # block (needed together by its accumulation groups) land back-to-back
        feats = [[None] * BR for _ in QBLOCKS]
        col = 0
        for ib, nqb in enumerate(QBLOCKS):
            nidx = nqb * 128
            for br in range(BR):
                ft = fpool.tile([128, nqb * B], bf16, name=f"f{ib}_{br}")
                dst3 = ft[:].rearrange("p (r e) -> p r e", e=B)
                nc.gpsimd.dma_gather(dst3, xg_d.ap()[br],
                                     idxt[:, col:col + nidx // 16],
                                     nidx, nidx, B)
                feats[ib][br] = ft
                col += nidx // 16

        # conv + drain + store; drains alternate ACT/DVE
        drain_tick = 0
        qoff = 0
        for ib, nqb in enumerate(QBLOCKS):
            for oc in range(6):
                ost = opool.tile([128, nqb * B], bf16, name=f"o{ib}_{oc}")
                for (coff, csz) in QCHUNKS[ib]:
                    pc = psump.tile([128, csz * B], f32)
                    for br in range(BR):
                        lhsT = wtb[:, br * OC + oc * 128:
                                   br * OC + (oc + 1) * 128]
                        nc.tensor.matmul(
                            pc[:], lhsT,
                            feats[ib][br][:, coff * B:(coff + csz) * B],
                            start=(br == 0), stop=(br == BR - 1))
                    dst = ost[:, coff * B:(coff + csz) * B]
                    if drain_tick % 2 == 0:
                        nc.scalar.copy(dst, pc[:])
                    else:
                        nc.vector.tensor_copy(dst, pc[:])
                    drain_tick += 1
                nc.sync.dma_start(
                    out_d.ap()[oc * 128:(oc + 1) * 128,
                               qoff * B:(qoff + nqb) * B], ost[:])
            qoff += nqb

    nc.compile()
    return nc


def _prep_idx(idx, offh, offw):
    """Per-core gather descriptor index arrays [128, NIDX_COLS] int16."""
    idx = np.asarray(idx).astype(np.int64)      # [3, 128, 256]
    oh = np.asarray(offh).astype(np.int64)      # [3, 128]
    ow = np.asarray(offw).astype(np.int64)
    g = np.arange(G)
    out = []
    for core in range(N_CORES):
        q0 = Q0[core]
        rlo = q0 // 14
        cols = np.empty((16, NIDX_COLS), np.int16)
        col = 0
        qoff = 0
        for nqb in QBLOCKS:
            q = q0 + qoff + np.arange(nqb)
            qi, qj = q // 14, q % 14
            for br in range(BR):
                # p[g, q] = grid position picked for (group, output position)
                p = 16 * (oh[br, :, None] + qi[None, :]) + \
                    (ow[br, :, None] + qj[None, :])
                val = (p - 16 * rlo) * C + idx[br, g[:, None], p]
                assert val.min() >= 0 and val.max() < WP * C
                flat = val.T.reshape(-1)        # j = q_local*128 + g
                ncols = len(flat) // 16
                cols[:, col:col + ncols] = flat.reshape(ncols, 16).T
                col += ncols
            qoff += nqb
        out.append(np.tile(cols, (8, 1)))
    return out


def kernel(x1, x3, x5, W, idx, offh, offw):
    x1 = np.asarray(x1, dtype=np.float32)
    x3 = np.asarray(x3, dtype=np.float32)
    x5 = np.asarray(x5, dtype=np.float32)
    W = np.asarray(W, dtype=np.float32)
    assert x1.shape == (B, C, 16, 16)

    if "nc" not in _CACHE:
        _CACHE["nc"] = _build_program()
    nc = _CACHE["nc"]

    # xg[br, p, c, b] = x_br[b, c, p]  (pure relayout)
    stack = np.stack([x1.reshape(B, C, 256),
                      x3.reshape(B, C, 256),
                      x5.reshape(B, C, 256)])          # [br, b, c, p]
    xg = np.ascontiguousarray(stack.transpose(0, 3, 2, 1)).astype(
        ml_dtypes.bfloat16)                            # [br, 256, 512, 128]

    # wt[g, br*768 + o] = W[o, br*128 + g]
    wt = np.ascontiguousarray(
        W.reshape(OC, BR, G).transpose(2, 1, 0).reshape(G, BR * OC)
    ).astype(ml_dtypes.bfloat16)

    idxts = _prep_idx(idx, offh, offw)

    in_maps = []
    for core in range(N_CORES):
        rlo = Q0[core] // 14
        win = np.ascontiguousarray(
            xg[:, rlo * 16:rlo * 16 + WP]).reshape(BR, WP * C, B)
        in_maps.append({"xg": win, "idxt": idxts[core], "wt": wt})

    res = bass_utils.run_bass_kernel_spmd(nc, in_maps, list(range(N_CORES)))

    out = np.empty((B, OC, NQ), np.float32)
    for core in range(N_CORES):
        o = np.asarray(res.results[core]["out"]).astype(np.float32)
        o = o.reshape(OC, NQC, B).transpose(2, 0, 1)   # [b, o, q_local]
        out[:, :, Q0[core]:Q0[core] + NQC] = o
    return out.reshape(B, OC, 14, 14)
